# revision 4
# baseline (speedup 1.0000x reference)
"""Causal multi-head attention on 8 TRN2 NeuronCores.

Reference computation (fp32):
    q,k,v = x @ {Q,K,V}.T split into 16 heads of 64
    scores = q k^T / 8, causal mask, softmax
    out    = (attn @ v concat heads) @ W_o.T

Sharding: core c (0..7) takes batch b = c//4 and head group g = c%4
(heads 4g..4g+3, i.e. a 256-row slice of Q/K/V and a 256-column slice
of W_o). Each core produces a partial [T, D] output; the host sums the
4 partials per batch. No on-device collectives.

Per-core DRAM layout (host pre-packs everything so every matmul
contraction dim lands on SBUF partitions AND every SBUF tile loads with
ONE wide dma_start -- DMA trigger instructions cost ~650ns each on the
issuing engine queue, so few big triggers beat many small ones):
    xp    [128, 16384] x[b].T packed chunk-major then db-major:
                       xp[p, 4096c + 512db + t] = x[b].T[128db+p, 512c+t]
    wqp   [128, 2048]  wqp[p, 256db+e] = Q[slice].T[128db+p, e]
    wkp   [128, 2048]  same for K
    wvp   [128, 2080]  V with a zero column after each head (260/db);
                       the ones-column, added via a rank-1 matmul, makes
                       the PV matmul emit the softmax denominator in
                       row 64 for free
    wop   [128, 2048]  wop[p, 1024db+e] = W_o[:, slice].T[128db+p, e]
    maskz [128, 128]   triangular f >= p mask ([tk, tq] orientation)
    ones [1, 128], wv1 [1, 260] (1 at each head's ones-column)

Input DMA triggers are split across the two HWDGE queues (Sync + ACT)
so the queues stream in parallel from t~8us.

Attention is computed transposed (ST[tk, tq] = k-block . qT-chunk) so
softmax exp is elementwise (no max subtraction: scores ~ N(0,1), exp
cannot overflow) and PV needs no transposes; exp runs on ACT straight
out of PSUM. The schedule keeps the PE stream dense so the HAM clock
gate stays at K=8/8 (idle gaps > ~3us re-throttle the PE to half
clock): PV lags ST by two j-steps (hiding exp latency), stage-1's
second half fills the c=0 attention chunk, and stage-5 fills the c=1
chunk. Softmax normalization runs off the PE critical path and reads
the PV accumulator directly from PSUM (reciprocal of the denominator
row in-place at partition 64, GpSimd partition-broadcast, multiply).
"""

import numpy as np

import concourse.bass as bass  # noqa: F401
import concourse.tile as tile
from concourse import bacc, mybir
from concourse.bass_utils import run_bass_kernel_spmd

F32 = mybir.dt.float32
F32R = mybir.dt.float32r
BF16 = mybir.dt.bfloat16
EXP = mybir.ActivationFunctionType.Exp

import os as _os

# matmul operand dtype: bf16 (full PE rate + fast weight load; all
# accumulations stay in fp32 PSUM and softmax denominators are computed
# in fp32, so the only loss is bf16 input/intermediate rounding,
# ~4e-3 relative). Set MHA_DTYPE=f32r for ~2e-4 at ~20% more time.
WDT = BF16 if _os.environ.get("MHA_DTYPE", "bf16") == "bf16" else F32R

N_CORES = 8
T = 2048          # sequence length
D = 1024          # model dim
HPC = 4           # heads per core
HD = 64           # head dim
DS = HPC * HD     # 256: per-core slice of D
VW = HPC * (HD + 1)  # 260: v tiles with ones-column per head
CH = 1024         # tq chunk width
NCH = T // CH     # chunks
NTB = T // 128    # 128-row t blocks
NDB = D // 128    # 128-row d blocks


def build_program():
    nc = bacc.Bacc("TRN2", target_bir_lowering=False, debug=False,
                   num_devices=N_CORES)
    xp_d = nc.dram_tensor("xp", [128, 16384], WDT, kind="ExternalInput").ap()
    wqp_d = nc.dram_tensor("wqp", [128, 2048], WDT, kind="ExternalInput").ap()
    wkp_d = nc.dram_tensor("wkp", [128, 2048], WDT, kind="ExternalInput").ap()
    wvp_d = nc.dram_tensor("wvp", [128, 8 * VW], WDT,
                           kind="ExternalInput").ap()
    wop_d = nc.dram_tensor("wop", [128, 2048], WDT, kind="ExternalInput").ap()
    maskz_d = nc.dram_tensor("maskz", [128, 128], WDT,
                             kind="ExternalInput").ap()
    ones_d = nc.dram_tensor("ones", [1, 128], WDT, kind="ExternalInput").ap()
    wv1_d = nc.dram_tensor("wv1", [1, VW], WDT, kind="ExternalInput").ap()
    out_d = nc.dram_tensor("out", [T, D], F32, kind="ExternalOutput").ap()

    with tile.TileContext(nc) as tc, \
         tc.tile_pool(name="xt", bufs=1) as xt_pool, \
         tc.tile_pool(name="wq", bufs=1) as wq_pool, \
         tc.tile_pool(name="wk", bufs=1) as wk_pool, \
         tc.tile_pool(name="wv", bufs=1) as wv_pool, \
         tc.tile_pool(name="wo", bufs=1) as wo_pool, \
         tc.tile_pool(name="cst", bufs=1) as cst_pool, \
         tc.tile_pool(name="qk", bufs=16) as qk_pool, \
         tc.tile_pool(name="vv", bufs=16) as vv_pool, \
         tc.tile_pool(name="ot", bufs=4) as ot_pool, \
         tc.tile_pool(name="ee", bufs=4) as e_pool, \
         tc.tile_pool(name="rd", bufs=2) as rd_pool, \
         tc.tile_pool(name="rb", bufs=2) as rb_pool, \
         tc.tile_pool(name="ob", bufs=4) as ob_pool:

        # ---- tiny constants first (needed by stage-1 v matmuls) --------
        ones_t = cst_pool.tile([1, 128], WDT, tag="ones")
        nc.sync.dma_start(ones_t[:], ones_d[:])
        wv1_t = cst_pool.tile([1, VW], WDT, tag="wv1")
        nc.sync.dma_start(wv1_t[:], wv1_d[:])

        # ---- input DMAs: few wide triggers, split across both HWDGE
        # queues (sync + scalar) so all streams are in flight by ~10us.
        # sync gets what stage-1 chunk 0 needs first.
        wq_t = wq_pool.tile([128, 2048], WDT, tag="wq")
        nc.sync.dma_start(wq_t[:], wqp_d[:])
        xt_t = xt_pool.tile([128, 16384], WDT, tag="xt")
        nc.sync.dma_start(xt_t[:, 0:2048], xp_d[:, 0:2048])
        nc.sync.dma_start(xt_t[:, 2048:4096], xp_d[:, 2048:4096])
        wk_t = wk_pool.tile([128, 2048], WDT, tag="wk")
        nc.sync.dma_start(wk_t[:], wkp_d[:])
        wv_t = wv_pool.tile([128, 8 * VW], WDT, tag="wv")
        nc.sync.dma_start(wv_t[:], wvp_d[:])

        maskz_t = cst_pool.tile([128, 128], WDT, tag="maskz")
        nc.scalar.dma_start(maskz_t[:], maskz_d[:])
        nc.scalar.dma_start(xt_t[:, 4096:8192], xp_d[:, 4096:8192])
        wo_t = wo_pool.tile([128, 2048], WDT, tag="wo")
        nc.scalar.dma_start(wo_t[:], wop_d[:])
        nc.scalar.dma_start(xt_t[:, 8192:12288], xp_d[:, 8192:12288])
        nc.scalar.dma_start(xt_t[:, 12288:16384], xp_d[:, 12288:16384])

        def xt_s(tch, db):  # x chunk tch, 128-row d block db: [128, 512]
            o = 4096 * tch + 512 * db
            return xt_t[:, o:o + 512]

        def wq_s(db):
            return wq_t[:, 256 * db:256 * db + 256]

        def wk_s(db):
            return wk_t[:, 256 * db:256 * db + 256]

        def wv_s(db):
            return wv_t[:, VW * db:VW * db + VW]

        def wo_s(db):
            return wo_t[:, 1024 * db:1024 * db + 1024]

        # ---- persistent E tiles (PV only reads exp-written regions) ----
        e_tiles = [e_pool.tile([128, CH], WDT, tag="ee", name=f"ee{i}")
                   for i in range(4)]

        # ---- stage 1: projections qT, kT (e on partitions), v (natural)
        qT_t = [[None] * 4 for _ in range(2)]
        kT_t = [[None] * 4 for _ in range(2)]
        v_t = [None] * NTB

        # oT_t[db][c]: [128, CH] attention outputs, d on partitions
        # (head h lives in tile h//2 rows 64*(h%2)..+64)
        oT_t = [[ot_pool.tile([128, CH], WDT, tag="ot", name=f"ot{d}_{c}")
                 for c in range(NCH)] for d in range(2)]
        state = {"eidx": 0}

        with tc.tile_pool(name="pst", bufs=2, space="PSUM") as pst_pool, \
             tc.tile_pool(name="pac", bufs=1, space="PSUM") as pac_pool:

            def emit_qk_group(ps1_pool, tch, eb, wsl, dst):
                ps = ps1_pool.tile([128, 512], F32, tag="ps1",
                                   name=f"p1_{tch}_{eb}_{dst is kT_t}")
                for db in range(NDB):
                    nc.tensor.matmul(
                        ps[:], wsl(db)[:, 128 * eb:128 * eb + 128],
                        xt_s(tch, db), start=(db == 0), stop=(db == NDB - 1))
                q = qk_pool.tile([128, 512], WDT, tag="qk",
                                 name=f"qk_{tch}_{eb}_{dst is kT_t}")
                nc.vector.tensor_copy(q[:], ps[:])
                dst[eb][tch] = q

            def emit_v_group(ps1_pool, tb):
                ps = ps1_pool.tile([128, VW], F32, tag="ps1",
                                   name=f"p1v_{tb}")
                for db in range(NDB):
                    nc.tensor.matmul(
                        ps[:],
                        xt_s(tb // 4, db)[:, 128 * (tb % 4):128 * (tb % 4) + 128],
                        wv_s(db), start=(db == 0), stop=False)
                # ones-columns: rank-1 update 1s^T . wv1
                nc.tensor.matmul(ps[:], ones_t[:], wv1_t[:],
                                 start=False, stop=True)
                v = vv_pool.tile([128, VW], WDT, tag="vv", name=f"v{tb}")
                nc.vector.tensor_copy(v[:], ps[:])
                v_t[tb] = v

            def emit_stage1_tch(ps1_pool, tch):
                for wsl, dst in ((wq_s, qT_t), (wk_s, kT_t)):
                    for eb in range(2):
                        emit_qk_group(ps1_pool, tch, eb, wsl, dst)
                for tb in range(4 * tch, 4 * tch + 4):
                    emit_v_group(ps1_pool, tb)

            def emit_pv(acc, c, h, j, e, off):
                jmax = 8 * c + 7
                alg = (off // 512) * 512
                for s in range(alg, CH, 512):
                    lo = max(s, off)
                    nc.tensor.matmul(
                        acc[:, lo:s + 512],
                        v_t[j][:, 65 * h:65 * h + 65],
                        e[:, lo:s + 512],
                        start=(j == 0),
                        stop=(j == (8 * c + 3 if s == 0 else jmax)),
                    )

            def emit_pair(c, h):
                # attention for one (chunk, head), PV delayed 2 j-steps so
                # the exp (ACT) latency never stalls the PE stream
                pb, rw = h // 2, 64 * (h % 2)
                jmax = 8 * c + 7
                acc = pac_pool.tile([65, CH], F32, tag="pac",
                                    name=f"ac{c}_{h}")
                pending = []
                for j in range(jmax + 1):
                    off = max(0, 128 * j - CH * c)
                    alg = (off // 512) * 512  # 512-aligned ST psum base
                    st = pst_pool.tile([128, CH], F32, tag="pst",
                                       name=f"st{c}_{h}_{j}")
                    # ST[tk, tq] = k-block . qT-chunk
                    for s in range(alg, CH, 512):
                        lo = max(s, off)
                        nc.tensor.matmul(
                            st[:, lo:s + 512],
                            kT_t[pb][j // 4][
                                rw:rw + 64,
                                128 * (j % 4):128 * (j % 4) + 128],
                            qT_t[pb][2 * c + s // 512][rw:rw + 64,
                                                       lo - s:512],
                            start=True, stop=True)
                    e = e_tiles[state["eidx"] % len(e_tiles)]
                    state["eidx"] += 1
                    nc.scalar.activation(e[:, off:], st[:, off:], EXP,
                                         scale=0.125)
                    if 128 * j >= CH * c:
                        # diagonal block: tri mask (PV reads from off on,
                        # so below-diagonal cols never need zeroing)
                        nc.vector.tensor_mul(
                            e[:, off:off + 128], e[:, off:off + 128],
                            maskz_t[:, 0:128])
                    pending.append((j, e, off))
                    if len(pending) > 2:
                        jd, ed, ad = pending.pop(0)
                        emit_pv(acc, c, h, jd, ed, ad)
                for jd, ed, ad in pending:
                    emit_pv(acc, c, h, jd, ed, ad)
                # normalization, off the PE critical path: copy the
                # denominator row down to partition 0 via sbuf-sbuf DMA,
                # approx-reciprocal, GpSimd partition-broadcast, then
                # multiply the PV accumulator (read from PSUM) by it.
                den64 = rd_pool.tile([65, CH], F32, tag="d64")
                nc.vector.tensor_copy(den64[64:65, :], acc[64:65, :])
                den0 = rd_pool.tile([1, CH], F32, tag="dn")
                nc.sync.dma_start(den0[:], den64[64:65, :])
                rden = rd_pool.tile([1, CH], F32, tag="rd")
                nc.vector.reciprocal_approx_fast(rden[:], den0[:])
                rbt = rb_pool.tile([128, CH], F32, tag="rb")
                for s in range(0, CH, 512):
                    nc.gpsimd.partition_broadcast(rbt[:, s:s + 512],
                                                  rden[:, s:s + 512])
                    nc.vector.tensor_mul(
                        oT_t[pb][c][rw:rw + 64, s:s + 512],
                        acc[0:64, s:s + 512], rbt[0:64, s:s + 512])

            with tc.tile_pool(name="ps1", bufs=2, space="PSUM") as ps1_pool:
                emit_stage1_tch(ps1_pool, 0)
                emit_stage1_tch(ps1_pool, 1)
                # c=0 attention interleaved with the rest of stage 1:
                # stage-1 matmul groups keep the PE dense while ACT
                # works through the exp stream
                fillers = ([("qk", 2, eb, wsl, dst)
                            for wsl, dst in ((wq_s, qT_t), (wk_s, kT_t))
                            for eb in range(2)]
                           + [("v", tb) for tb in range(8, 12)]
                           + [("qk", 3, eb, wsl, dst)
                              for wsl, dst in ((wq_s, qT_t), (wk_s, kT_t))
                              for eb in range(2)]
                           + [("v", tb) for tb in range(12, 16)])
                for h in range(HPC):
                    emit_pair(0, h)
                    for f in fillers[4 * h:4 * h + 4]:
                        if f[0] == "qk":
                            emit_qk_group(ps1_pool, f[1], f[2], f[3], f[4])
                        else:
                            emit_v_group(ps1_pool, f[1])

            def emit_stage5(ps5_pool, tb, cp_engines):
                c, tw = tb // 8, 128 * (tb % 8)
                pss = [ps5_pool.tile([128, 512], F32, tag="ps5",
                                     name=f"ps5_{tb}_{eb}")
                       for eb in range(2)]
                for db in range(2):
                    for eb in range(2):
                        nc.tensor.matmul(
                            pss[eb][:], oT_t[db][c][:, tw:tw + 128],
                            wo_s(db)[:, 512 * eb:512 * eb + 512],
                            start=(db == 0), stop=(db == 1))
                ob = ob_pool.tile([128, 1024], F32, tag="ob",
                                  name=f"ob{tb}")
                for eb in range(2):
                    cp_engines[eb](ob[:, 512 * eb:512 * eb + 512],
                                   pss[eb][:])
                nc.sync.dma_start(out_d[128 * tb:128 * tb + 128, :], ob[:])

            # c=1 attention interleaved with stage-5 on the finished c=0
            # chunk. ACT is mid-exp-stream here, so psum->sbuf copies
            # stay on DVE.
            with tc.tile_pool(name="ps5", bufs=2, space="PSUM") as ps5_pool:
                cpv = (nc.vector.tensor_copy, nc.vector.tensor_copy)
                for h in range(HPC):
                    emit_pair(1, h)
                    emit_stage5(ps5_pool, 2 * h, cpv)
                    emit_stage5(ps5_pool, 2 * h + 1, cpv)

        # tail: exp stream is done, so ACT picks up half the copies, and
        # a deeper psum pool (pst/pac closed above) keeps two t-blocks
        # in flight.
        with tc.tile_pool(name="ps5b", bufs=4, space="PSUM") as ps5b_pool:
            cps = (nc.vector.tensor_copy, nc.scalar.copy)
            for tb in range(8, NTB):
                emit_stage5(ps5b_pool, tb, cps)

    nc.compile()
    return nc


_PROG = None


def _get_prog():
    global _PROG
    if _PROG is None:
        _PROG = build_program()
    return _PROG


def make_in_maps(x, Q, K, V, W_o):
    np_dt = mybir.dt.np(WDT)
    B = x.shape[0]
    maskz = np.greater_equal(np.arange(128)[None, :],
                             np.arange(128)[:, None]).astype(np.float32)
    maskz = maskz.astype(np_dt)
    ones = np.ones((1, 128), dtype=np_dt)
    wv1 = np.zeros((1, VW), dtype=np.float32)
    wv1[0, 64::65] = 1.0
    wv1 = wv1.astype(np_dt)

    def pack_rows(w, cols):  # [1024, cols] -> [128, 8*cols]
        return np.ascontiguousarray(
            w.reshape(8, 128, cols).transpose(1, 0, 2).reshape(128, 8 * cols))

    in_maps = []
    for c in range(N_CORES):
        b, g = divmod(c, N_CORES // B)
        sl = slice(DS * g, DS * g + DS)
        wvT = V[sl, :].T  # [D, 256]
        wvT_pad = np.zeros((D, VW), dtype=np.float32)
        for h in range(HPC):
            wvT_pad[:, 65 * h:65 * h + 64] = wvT[:, 64 * h:64 * h + 64]
        xT = np.ascontiguousarray(x[b].T)  # [1024, 2048]
        # xp[p, 4096c + 512db + t] = xT[128db+p, 512c+t]
        xp = (xT.reshape(8, 128, 4, 512).transpose(1, 2, 0, 3)
              .reshape(128, 16384))
        wop = W_o[:, sl].T  # [256, 1024]
        wop = (wop.reshape(2, 128, 1024).transpose(1, 0, 2)
               .reshape(128, 2048))
        in_maps.append({
            "xp": np.ascontiguousarray(xp).astype(np_dt),
            "wqp": pack_rows(Q[sl, :].T, DS).astype(np_dt),
            "wkp": pack_rows(K[sl, :].T, DS).astype(np_dt),
            "wvp": pack_rows(wvT_pad, VW).astype(np_dt),
            "wop": np.ascontiguousarray(wop).astype(np_dt),
            "maskz": maskz,
            "ones": ones,
            "wv1": wv1,
        })
    return in_maps


def kernel(x, Q, K, V, W_o):
    x = np.asarray(x, dtype=np.float32)
    Q = np.asarray(Q, dtype=np.float32)
    K = np.asarray(K, dtype=np.float32)
    V = np.asarray(V, dtype=np.float32)
    W_o = np.asarray(W_o, dtype=np.float32)

    nc = _get_prog()
    in_maps = make_in_maps(x, Q, K, V, W_o)
    res = run_bass_kernel_spmd(nc, in_maps, core_ids=list(range(N_CORES)))

    B = x.shape[0]
    out = np.zeros((B, T, D), dtype=np.float32)
    for c in range(N_CORES):
        out[c // (N_CORES // B)] += res.results[c]["out"]
    return out


# revision 7
# speedup vs baseline: 1.1563x; 1.1563x over previous
"""Causal multi-head attention on 8 TRN2 NeuronCores.

Reference computation (fp32):
    q,k,v = x @ {Q,K,V}.T split into 16 heads of 64
    scores = q k^T / 8, causal mask, softmax
    out    = (attn @ v concat heads) @ W_o.T

Sharding: core c (0..7) takes batch b = c//4 and head group g = c%4
(heads 4g..4g+3, i.e. a 256-row slice of Q/K/V and a 256-column slice
of W_o). Each core produces a partial [T, D] output; the host sums the
4 partials per batch. No on-device collectives.

Per-core DRAM layout (host pre-packs everything so every matmul
contraction dim lands on SBUF partitions AND every SBUF tile loads with
ONE wide dma_start -- DMA trigger instructions cost ~650ns each on the
issuing engine queue, so few big triggers beat many small ones):
    xp    [128, 16384] x[b].T packed chunk-major then db-major:
                       xp[p, 4096c + 512db + t] = x[b].T[128db+p, 512c+t]
    wqp   [128, 2048]  wqp[p, 256db+e] = Q[slice].T[128db+p, e]
    wkp   [128, 2048]  same for K
    wvp   [128, 2080]  V with a zero column after each head (260/db);
                       the ones-column, added via a rank-1 matmul, makes
                       the PV matmul emit the softmax denominator in
                       row 64 for free
    wop   [128, 2048]  wop[p, 1024db+e] = W_o[:, slice].T[128db+p, e]
    maskz [128, 128]   triangular f >= p mask ([tk, tq] orientation)
    ones [1, 128], wv1 [1, 260] (1 at each head's ones-column)

Input DMA triggers are split across the two HWDGE queues (Sync + ACT)
so the queues stream in parallel from t~8us.

Attention is computed transposed (ST[tk, tq] = k-block . qT-chunk) so
softmax exp is elementwise (no max subtraction: scores ~ N(0,1), exp
cannot overflow) and PV needs no transposes; exp runs on ACT straight
out of PSUM. The schedule keeps the PE stream dense so the HAM clock
gate stays at K=8/8 (idle gaps > ~3us re-throttle the PE to half
clock): PV lags ST by two j-steps (hiding exp latency), stage-1's
second half fills the c=0 attention chunk, and stage-5 fills the c=1
chunk. Softmax normalization runs off the PE critical path and reads
the PV accumulator directly from PSUM (reciprocal of the denominator
row in-place at partition 64, GpSimd partition-broadcast, multiply).
"""

import numpy as np

import concourse.bass as bass  # noqa: F401
import concourse.tile as tile
from concourse import bacc, mybir
from concourse.bass_utils import run_bass_kernel_spmd

F32 = mybir.dt.float32
F32R = mybir.dt.float32r
BF16 = mybir.dt.bfloat16
EXP = mybir.ActivationFunctionType.Exp

import os as _os

# matmul operand dtype: bf16 (full PE rate + fast weight load; all
# accumulations stay in fp32 PSUM and softmax denominators are computed
# in fp32, so the only loss is bf16 input/intermediate rounding,
# ~4e-3 relative). Set MHA_DTYPE=f32r for ~2e-4 at ~20% more time.
WDT = BF16 if _os.environ.get("MHA_DTYPE", "bf16") == "bf16" else F32R

N_CORES = 8
T = 2048          # sequence length
D = 1024          # model dim
HPC = 4           # heads per core
HD = 64           # head dim
DS = HPC * HD     # 256: per-core slice of D
VW = HPC * (HD + 1)  # 260: v tiles with ones-column per head
CH = 1024         # tq chunk width
NCH = T // CH     # chunks
NTB = T // 128    # 128-row t blocks
NDB = D // 128    # 128-row d blocks


def build_program():
    nc = bacc.Bacc("TRN2", target_bir_lowering=False, debug=False,
                   num_devices=N_CORES)
    xp_d = nc.dram_tensor("xp", [128, 16384], WDT, kind="ExternalInput").ap()
    wqp_d = nc.dram_tensor("wqp", [128, 2048], WDT, kind="ExternalInput").ap()
    wkp_d = nc.dram_tensor("wkp", [128, 2048], WDT, kind="ExternalInput").ap()
    wvp_d = nc.dram_tensor("wvp", [128, 8 * VW], WDT,
                           kind="ExternalInput").ap()
    wop_d = nc.dram_tensor("wop", [128, 2048], WDT, kind="ExternalInput").ap()
    maskz_d = nc.dram_tensor("maskz", [128, 128], WDT,
                             kind="ExternalInput").ap()
    ones_d = nc.dram_tensor("ones", [1, 128], WDT, kind="ExternalInput").ap()
    wv1_d = nc.dram_tensor("wv1", [1, VW], WDT, kind="ExternalInput").ap()
    out_d = nc.dram_tensor("out", [T, D], F32, kind="ExternalOutput").ap()

    with tile.TileContext(nc) as tc, \
         tc.tile_pool(name="xt", bufs=1) as xt_pool, \
         tc.tile_pool(name="wq", bufs=1) as wq_pool, \
         tc.tile_pool(name="wk", bufs=1) as wk_pool, \
         tc.tile_pool(name="wv", bufs=1) as wv_pool, \
         tc.tile_pool(name="wo", bufs=1) as wo_pool, \
         tc.tile_pool(name="cst", bufs=1) as cst_pool, \
         tc.tile_pool(name="qk", bufs=16) as qk_pool, \
         tc.tile_pool(name="vv", bufs=16) as vv_pool, \
         tc.tile_pool(name="ot", bufs=4) as ot_pool, \
         tc.tile_pool(name="ee", bufs=4) as e_pool, \
         tc.tile_pool(name="rd", bufs=2) as rd_pool, \
         tc.tile_pool(name="ou", bufs=2) as ou_pool, \
         tc.tile_pool(name="rb", bufs=2) as rb_pool, \
         tc.tile_pool(name="ob", bufs=4) as ob_pool:

        # ---- tiny constants first (needed by stage-1 v matmuls) --------
        ones_t = cst_pool.tile([1, 128], WDT, tag="ones")
        nc.sync.dma_start(ones_t[:], ones_d[:])
        wv1_t = cst_pool.tile([1, VW], WDT, tag="wv1")
        nc.sync.dma_start(wv1_t[:], wv1_d[:])

        # ---- input DMAs: wide triggers split across both HWDGE queues
        # (sync + scalar) so all streams are in flight by ~11us. The
        # sync queue carries what stage-1 chunk 0 consumes, in
        # consumption order and with small first bites so the first
        # matmul group unblocks as early as possible; the scalar queue
        # (idle until the exp stream starts) carries the rest.
        wq_t = wq_pool.tile([128, 2048], WDT, tag="wq")
        xt_t = xt_pool.tile([128, 16384], WDT, tag="xt")
        nc.sync.dma_start(wq_t[:, 0:512], wqp_d[:, 0:512])
        nc.sync.dma_start(xt_t[:, 0:1024], xp_d[:, 0:1024])
        nc.sync.dma_start(wq_t[:, 512:2048], wqp_d[:, 512:2048])
        nc.sync.dma_start(xt_t[:, 1024:4096], xp_d[:, 1024:4096])
        wk_t = wk_pool.tile([128, 2048], WDT, tag="wk")
        nc.sync.dma_start(wk_t[:], wkp_d[:])
        wv_t = wv_pool.tile([128, 8 * VW], WDT, tag="wv")
        nc.sync.dma_start(wv_t[:], wvp_d[:])

        maskz_t = cst_pool.tile([128, 128], WDT, tag="maskz")
        nc.scalar.dma_start(maskz_t[:], maskz_d[:])
        nc.scalar.dma_start(xt_t[:, 4096:8192], xp_d[:, 4096:8192])
        wo_t = wo_pool.tile([128, 2048], WDT, tag="wo")
        nc.scalar.dma_start(wo_t[:], wop_d[:])
        nc.scalar.dma_start(xt_t[:, 8192:12288], xp_d[:, 8192:12288])
        nc.scalar.dma_start(xt_t[:, 12288:16384], xp_d[:, 12288:16384])

        def xt_s(tch, db):  # x chunk tch, 128-row d block db: [128, 512]
            o = 4096 * tch + 512 * db
            return xt_t[:, o:o + 512]

        def wq_s(db):
            return wq_t[:, 256 * db:256 * db + 256]

        def wk_s(db):
            return wk_t[:, 256 * db:256 * db + 256]

        def wv_s(db):
            return wv_t[:, VW * db:VW * db + VW]

        def wo_s(db):
            return wo_t[:, 1024 * db:1024 * db + 1024]

        # ---- persistent E tiles (PV only reads exp-written regions) ----
        e_tiles = [e_pool.tile([128, CH], WDT, tag="ee", name=f"ee{i}")
                   for i in range(4)]

        # ---- stage 1: projections qT, kT (e on partitions), v (natural)
        qT_t = [[None] * 4 for _ in range(2)]
        kT_t = [[None] * 4 for _ in range(2)]
        v_t = [None] * NTB

        # oT_t[db][c]: [128, CH] attention outputs, d on partitions
        # (head h lives in tile h//2 rows 64*(h%2)..+64)
        oT_t = [[ot_pool.tile([128, CH], WDT, tag="ot", name=f"ot{d}_{c}")
                 for c in range(NCH)] for d in range(2)]
        state = {"eidx": 0}

        with tc.tile_pool(name="pst", bufs=2, space="PSUM") as pst_pool, \
             tc.tile_pool(name="pac", bufs=1, space="PSUM") as pac_pool:

            def emit_qk_group(ps1_pool, tch, eb, wsl, dst):
                ps = ps1_pool.tile([128, 512], F32, tag="ps1",
                                   name=f"p1_{tch}_{eb}_{dst is kT_t}")
                for db in range(NDB):
                    nc.tensor.matmul(
                        ps[:], wsl(db)[:, 128 * eb:128 * eb + 128],
                        xt_s(tch, db), start=(db == 0), stop=(db == NDB - 1))
                q = qk_pool.tile([128, 512], WDT, tag="qk",
                                 name=f"qk_{tch}_{eb}_{dst is kT_t}")
                nc.vector.tensor_copy(q[:], ps[:])
                dst[eb][tch] = q

            def emit_v_group(ps1_pool, tb):
                ps = ps1_pool.tile([128, VW], F32, tag="ps1",
                                   name=f"p1v_{tb}")
                for db in range(NDB):
                    nc.tensor.matmul(
                        ps[:],
                        xt_s(tb // 4, db)[:, 128 * (tb % 4):128 * (tb % 4) + 128],
                        wv_s(db), start=(db == 0), stop=False)
                # ones-columns: rank-1 update 1s^T . wv1
                nc.tensor.matmul(ps[:], ones_t[:], wv1_t[:],
                                 start=False, stop=True)
                v = vv_pool.tile([128, VW], WDT, tag="vv", name=f"v{tb}")
                nc.vector.tensor_copy(v[:], ps[:])
                v_t[tb] = v

            def emit_stage1_tch(ps1_pool, tch):
                for wsl, dst in ((wq_s, qT_t), (wk_s, kT_t)):
                    for eb in range(2):
                        emit_qk_group(ps1_pool, tch, eb, wsl, dst)
                for tb in range(4 * tch, 4 * tch + 4):
                    emit_v_group(ps1_pool, tb)

            def emit_pv(acc, c, h, j, e, off):
                jmax = 8 * c + 7
                alg = (off // 512) * 512
                for s in range(alg, CH, 512):
                    lo = max(s, off)
                    nc.tensor.matmul(
                        acc[:, lo:s + 512],
                        v_t[j][:, 65 * h:65 * h + 65],
                        e[:, lo:s + 512],
                        start=(j == 0),
                        stop=(j == (8 * c + 3 if s == 0 else jmax)),
                    )

            def emit_pair(c, h):
                # attention for one (chunk, head), PV delayed 2 j-steps so
                # the exp (ACT) latency never stalls the PE stream
                pb, rw = h // 2, 64 * (h % 2)
                jmax = 8 * c + 7
                acc = pac_pool.tile([65, CH], F32, tag="pac",
                                    name=f"ac{c}_{h}")
                pending = []
                for j in range(jmax + 1):
                    off = max(0, 128 * j - CH * c)
                    alg = (off // 512) * 512  # 512-aligned ST psum base
                    st = pst_pool.tile([128, CH], F32, tag="pst",
                                       name=f"st{c}_{h}_{j}")
                    # ST[tk, tq] = k-block . qT-chunk
                    for s in range(alg, CH, 512):
                        lo = max(s, off)
                        nc.tensor.matmul(
                            st[:, lo:s + 512],
                            kT_t[pb][j // 4][
                                rw:rw + 64,
                                128 * (j % 4):128 * (j % 4) + 128],
                            qT_t[pb][2 * c + s // 512][rw:rw + 64,
                                                       lo - s:512],
                            start=True, stop=True)
                    e = e_tiles[state["eidx"] % len(e_tiles)]
                    state["eidx"] += 1
                    nc.scalar.activation(e[:, off:], st[:, off:], EXP,
                                         scale=0.125)
                    if 128 * j >= CH * c:
                        # diagonal block: tri mask (PV reads from off on,
                        # so below-diagonal cols never need zeroing)
                        nc.vector.tensor_mul(
                            e[:, off:off + 128], e[:, off:off + 128],
                            maskz_t[:, 0:128])
                    pending.append((j, e, off))
                    if len(pending) > 2:
                        jd, ed, ad = pending.pop(0)
                        emit_pv(acc, c, h, jd, ed, ad)
                for jd, ed, ad in pending:
                    emit_pv(acc, c, h, jd, ed, ad)
                # normalization, entirely off the PE critical path: one
                # whole-acc copy frees PSUM fast (so the next pair's PV
                # never waits on this chain), then approx-reciprocal of
                # the denominator row, GpSimd partition-broadcast, mul.
                oTu = ou_pool.tile([65, CH], F32, tag="ou")
                nc.vector.tensor_copy(oTu[:], acc[:])
                den0 = rd_pool.tile([1, CH], F32, tag="dn")
                nc.sync.dma_start(den0[:], oTu[64:65, :])
                rden = rd_pool.tile([1, CH], F32, tag="rd")
                nc.vector.reciprocal_approx_fast(rden[:], den0[:])
                rbt = rb_pool.tile([128, CH], F32, tag="rb")
                for s in range(0, CH, 512):
                    nc.gpsimd.partition_broadcast(rbt[:, s:s + 512],
                                                  rden[:, s:s + 512])
                    nc.vector.tensor_mul(
                        oT_t[pb][c][rw:rw + 64, s:s + 512],
                        oTu[0:64, s:s + 512], rbt[0:64, s:s + 512])

            with tc.tile_pool(name="ps1", bufs=2, space="PSUM") as ps1_pool:
                emit_stage1_tch(ps1_pool, 0)
                emit_stage1_tch(ps1_pool, 1)
                # c=0 attention interleaved with the rest of stage 1:
                # stage-1 matmul groups keep the PE dense while ACT
                # works through the exp stream
                fillers = ([("qk", 2, eb, wsl, dst)
                            for wsl, dst in ((wq_s, qT_t), (wk_s, kT_t))
                            for eb in range(2)]
                           + [("v", tb) for tb in range(8, 12)]
                           + [("qk", 3, eb, wsl, dst)
                              for wsl, dst in ((wq_s, qT_t), (wk_s, kT_t))
                              for eb in range(2)]
                           + [("v", tb) for tb in range(12, 16)])
                for h in range(HPC):
                    emit_pair(0, h)
                    for f in fillers[4 * h:4 * h + 4]:
                        if f[0] == "qk":
                            emit_qk_group(ps1_pool, f[1], f[2], f[3], f[4])
                        else:
                            emit_v_group(ps1_pool, f[1])

            def emit_stage5(ps5_pool, tb, cp_engines):
                c, tw = tb // 8, 128 * (tb % 8)
                pss = [ps5_pool.tile([128, 512], F32, tag="ps5",
                                     name=f"ps5_{tb}_{eb}")
                       for eb in range(2)]
                for db in range(2):
                    for eb in range(2):
                        nc.tensor.matmul(
                            pss[eb][:], oT_t[db][c][:, tw:tw + 128],
                            wo_s(db)[:, 512 * eb:512 * eb + 512],
                            start=(db == 0), stop=(db == 1))
                ob = ob_pool.tile([128, 1024], F32, tag="ob",
                                  name=f"ob{tb}")
                for eb in range(2):
                    cp_engines[eb](ob[:, 512 * eb:512 * eb + 512],
                                   pss[eb][:])
                nc.sync.dma_start(out_d[128 * tb:128 * tb + 128, :], ob[:])

            # c=1 attention interleaved with stage-5 on the finished c=0
            # chunk. ACT is mid-exp-stream here, so psum->sbuf copies
            # stay on DVE.
            with tc.tile_pool(name="ps5", bufs=2, space="PSUM") as ps5_pool:
                cpv = (nc.vector.tensor_copy, nc.vector.tensor_copy)
                for h in range(HPC):
                    emit_pair(1, h)
                    emit_stage5(ps5_pool, 2 * h, cpv)
                    emit_stage5(ps5_pool, 2 * h + 1, cpv)

        # tail: exp stream is done, so ACT picks up half the copies, and
        # a deeper psum pool (pst/pac closed above) keeps two t-blocks
        # in flight.
        with tc.tile_pool(name="ps5b", bufs=4, space="PSUM") as ps5b_pool:
            cps = (nc.vector.tensor_copy, nc.scalar.copy)
            for tb in range(8, NTB):
                emit_stage5(ps5b_pool, tb, cps)

    nc.compile()
    return nc


_PROG = None


def _get_prog():
    global _PROG
    if _PROG is None:
        _PROG = build_program()
    return _PROG


def make_in_maps(x, Q, K, V, W_o):
    np_dt = mybir.dt.np(WDT)
    B = x.shape[0]
    maskz = np.greater_equal(np.arange(128)[None, :],
                             np.arange(128)[:, None]).astype(np.float32)
    maskz = maskz.astype(np_dt)
    ones = np.ones((1, 128), dtype=np_dt)
    wv1 = np.zeros((1, VW), dtype=np.float32)
    wv1[0, 64::65] = 1.0
    wv1 = wv1.astype(np_dt)

    def pack_rows(w, cols):  # [1024, cols] -> [128, 8*cols]
        return np.ascontiguousarray(
            w.reshape(8, 128, cols).transpose(1, 0, 2).reshape(128, 8 * cols))

    in_maps = []
    for c in range(N_CORES):
        b, g = divmod(c, N_CORES // B)
        sl = slice(DS * g, DS * g + DS)
        wvT = V[sl, :].T  # [D, 256]
        wvT_pad = np.zeros((D, VW), dtype=np.float32)
        for h in range(HPC):
            wvT_pad[:, 65 * h:65 * h + 64] = wvT[:, 64 * h:64 * h + 64]
        xT = np.ascontiguousarray(x[b].T)  # [1024, 2048]
        # xp[p, 4096c + 512db + t] = xT[128db+p, 512c+t]
        xp = (xT.reshape(8, 128, 4, 512).transpose(1, 2, 0, 3)
              .reshape(128, 16384))
        wop = W_o[:, sl].T  # [256, 1024]
        wop = (wop.reshape(2, 128, 1024).transpose(1, 0, 2)
               .reshape(128, 2048))
        in_maps.append({
            "xp": np.ascontiguousarray(xp).astype(np_dt),
            "wqp": pack_rows(Q[sl, :].T, DS).astype(np_dt),
            "wkp": pack_rows(K[sl, :].T, DS).astype(np_dt),
            "wvp": pack_rows(wvT_pad, VW).astype(np_dt),
            "wop": np.ascontiguousarray(wop).astype(np_dt),
            "maskz": maskz,
            "ones": ones,
            "wv1": wv1,
        })
    return in_maps


def kernel(x, Q, K, V, W_o):
    x = np.asarray(x, dtype=np.float32)
    Q = np.asarray(Q, dtype=np.float32)
    K = np.asarray(K, dtype=np.float32)
    V = np.asarray(V, dtype=np.float32)
    W_o = np.asarray(W_o, dtype=np.float32)

    nc = _get_prog()
    in_maps = make_in_maps(x, Q, K, V, W_o)
    res = run_bass_kernel_spmd(nc, in_maps, core_ids=list(range(N_CORES)))

    B = x.shape[0]
    out = np.zeros((B, T, D), dtype=np.float32)
    for c in range(N_CORES):
        out[c // (N_CORES // B)] += res.results[c]["out"]
    return out
